# revision 2
# baseline (speedup 1.0000x reference)
"""Contrastive loss kernel for 8 TRN2 NeuronCores (Bass/Tile).

Algorithm (host sorts rows by class so same-class pairs are contiguous):
  loss*n = pos + neg
  pos = sum_c cnt_c^2 - sum_c ||v_c||^2            (v_c = class-sum embedding; tiny matmul)
  neg = sum_ij sim*[sim>m_i] - sum_{same ij} sim*[sim>m_i]
      = sum_i [ sum_j relu(sim-m_i) + m_i * cnt_i ] - (same-class correction over
        narrow sorted-class column windows)

Per core: 8 row-chunks x 16 col-chunks of [128,512] sim tiles (bf16 matmul, fp32 psum).
ScalarE does relu+row-accumulate (fused bias=-m), VectorE does is_gt+row-accumulate.
The same-class correction reuses identical matmul operand values so the threshold
counts cancel exactly.  Host does the final O(n) reduction in float64.
"""

import numpy as np
import ml_dtypes
from contextlib import ExitStack

import concourse.bacc as bacc
import concourse.mybir as mybir
import concourse.tile as tile
from concourse.bass_utils import run_bass_kernel_spmd

N, D, C = 8192, 128, 100
M = 8             # cores
RPC = N // M      # 1024 rows per core
NCH = RPC // 128  # 8 row-chunks per core
NJ = N // 512     # 16 col-chunks
W = 512           # correction window width

BF16 = ml_dtypes.bfloat16

_nc_cache = None
LAST_RESULTS = None


def _build_nc():
    f32 = mybir.dt.float32
    bf = mybir.dt.bfloat16
    A = mybir.ActivationFunctionType
    OP = mybir.AluOpType

    nc = bacc.Bacc("TRN2", target_bir_lowering=False, debug=False)

    xt = nc.dram_tensor("xt", [128, N], bf, kind="ExternalInput")        # X_sorted^T (full)
    xtl = nc.dram_tensor("xtl", [128, RPC], bf, kind="ExternalInput")    # core's rows, transposed
    xtw = nc.dram_tensor("xtw", [128, NCH * W], bf, kind="ExternalInput")  # correction windows
    xsr = nc.dram_tensor("xsr", [RPC, 128], bf, kind="ExternalInput")    # core's rows, untransposed
    mrow = nc.dram_tensor("mrow", [128, NCH], f32, kind="ExternalInput")
    trow = nc.dram_tensor("trow", [128, NCH], f32, kind="ExternalInput")
    eqm = nc.dram_tensor("eqm", [128, NCH * W], bf, kind="ExternalInput")
    iotab = nc.dram_tensor("iotab", [128, C], f32, kind="ExternalInput")
    out_acc = nc.dram_tensor("out_acc", [128, 32], f32, kind="ExternalOutput")
    out_v = nc.dram_tensor("out_v", [C, 128], f32, kind="ExternalOutput")

    with tile.TileContext(nc) as tc, ExitStack() as ctx:
        consts = ctx.enter_context(tc.tile_pool(name="consts", bufs=1))
        psum = ctx.enter_context(tc.tile_pool(name="psum", bufs=4, space="PSUM"))
        vpsum = ctx.enter_context(tc.tile_pool(name="vpsum", bufs=1, space="PSUM"))
        scratch = ctx.enter_context(tc.tile_pool(name="scratch", bufs=4))
        accp = ctx.enter_context(tc.tile_pool(name="accs", bufs=2))

        dma = nc.default_dma_engine

        xtl_sb = consts.tile([128, RPC], bf)
        dma.dma_start(out=xtl_sb[:], in_=xtl[:])
        xt_sb = consts.tile([128, N], bf)
        for p in range(4):
            s = p * (N // 4)
            dma.dma_start(out=xt_sb[:, s:s + N // 4], in_=xt[:, s:s + N // 4])
        m_sb = consts.tile([128, NCH], f32)
        dma.dma_start(out=m_sb[:], in_=mrow[:])
        t_sb = consts.tile([128, NCH], f32)
        dma.dma_start(out=t_sb[:], in_=trow[:])
        xtw_sb = consts.tile([128, NCH * W], bf)
        dma.dma_start(out=xtw_sb[:], in_=xtw[:])
        eqm_sb = consts.tile([128, NCH * W], bf)
        dma.dma_start(out=eqm_sb[:], in_=eqm[:])
        io_sb = consts.tile([128, C], f32)
        dma.dma_start(out=io_sb[:], in_=iotab[:])
        xs_sb = consts.tile([128, NCH, 128], bf)
        for ch in range(NCH):
            dma.dma_start(out=xs_sb[:, ch, :], in_=xsr[ch * 128:(ch + 1) * 128, :])

        negm = consts.tile([128, NCH], f32)
        nc.vector.tensor_scalar_mul(negm[:], m_sb[:], -1.0)

        st_all = consts.tile([128, NCH, C], bf)   # one-hot (class == target) per row-chunk
        oacc = accp.tile([128, 32], f32)

        for ch in range(NCH):
            lhsT = xtl_sb[:, ch * 128:(ch + 1) * 128]
            racc = accp.tile([128, NJ], f32, tag="racc")
            cacc = accp.tile([128, NJ], f32, tag="cacc")
            for j in range(NJ):
                ps = psum.tile([128, 512], mybir.dt.float32, tag="ps")
                nc.tensor.matmul(ps[:], lhsT, xt_sb[:, j * 512:(j + 1) * 512],
                                 start=True, stop=True)
                sA = scratch.tile([128, 512], bf, tag="sA")
                nc.scalar.activation(sA[:], ps[:], A.Relu,
                                     bias=negm[:, ch:ch + 1], scale=1.0,
                                     accum_out=racc[:, j:j + 1])
                sG = scratch.tile([128, 512], bf, tag="sG")
                nc.vector.tensor_scalar(sG[:], ps[:], m_sb[:, ch:ch + 1], None,
                                        OP.is_gt, OP.add,
                                        accum_out=cacc[:, j:j + 1])
            nc.vector.tensor_reduce(oacc[:, ch:ch + 1], racc[:],
                                    axis=mybir.AxisListType.X, op=OP.add)
            nc.vector.tensor_reduce(oacc[:, 8 + ch:9 + ch], cacc[:],
                                    axis=mybir.AxisListType.X, op=OP.add)

            # same-class correction over this row-chunk's sorted window
            psw = psum.tile([128, 512], mybir.dt.float32, tag="ps")
            nc.tensor.matmul(psw[:], lhsT, xtw_sb[:, ch * W:(ch + 1) * W],
                             start=True, stop=True)
            uw = scratch.tile([128, 512], bf, tag="sA")
            nc.scalar.activation(uw[:], psw[:], A.Relu,
                                 bias=negm[:, ch:ch + 1], scale=1.0)
            gw = scratch.tile([128, 512], bf, tag="sG")
            nc.vector.tensor_scalar(gw[:], psw[:], m_sb[:, ch:ch + 1], None, OP.is_gt)
            jk1 = scratch.tile([128, 512], bf, tag="jk")
            nc.vector.tensor_mul(jk1[:], eqm_sb[:, ch * W:(ch + 1) * W], uw[:])
            nc.vector.tensor_reduce(oacc[:, 16 + ch:17 + ch], jk1[:],
                                    axis=mybir.AxisListType.X, op=OP.add)
            jk2 = scratch.tile([128, 512], bf, tag="jk")
            nc.vector.tensor_mul(jk2[:], eqm_sb[:, ch * W:(ch + 1) * W], gw[:])
            nc.vector.tensor_reduce(oacc[:, 24 + ch:25 + ch], jk2[:],
                                    axis=mybir.AxisListType.X, op=OP.add)

            nc.vector.tensor_scalar(st_all[:, ch, :], io_sb[:], t_sb[:, ch:ch + 1],
                                    None, OP.is_equal)

        # partial class-sum vectors V_k = S^T X over this core's rows
        v_ps = vpsum.tile([C, 128], mybir.dt.float32)
        for ch in range(NCH):
            nc.tensor.matmul(v_ps[:], st_all[:, ch, :], xs_sb[:, ch, :],
                             start=(ch == 0), stop=(ch == NCH - 1))
        v_sb = accp.tile([C, 128], f32)
        nc.scalar.copy(v_sb[:], v_ps[:])

        dma.dma_start(out=out_v[:], in_=v_sb[:])
        dma.dma_start(out=out_acc[:], in_=oacc[:])

    nc.compile()
    return nc


def _prep(inputs, margin, targets):
    """Host-side sharding/layout prep. Returns per-core input maps + reduction data."""
    t = np.asarray(targets).astype(np.int64)
    x = np.asarray(inputs, dtype=np.float32)
    m = np.asarray(margin, dtype=np.float32)

    perm = np.argsort(t, kind="stable")
    xs, ms, ts = x[perm], m[perm], t[perm]
    x_bf = xs.astype(BF16)
    xt_bf = np.ascontiguousarray(x_bf.T)          # [128, N]

    cnt = np.bincount(ts, minlength=C).astype(np.float64)
    starts = np.concatenate([[0], np.cumsum(np.bincount(ts, minlength=C))]).astype(np.int64)

    nchunks = N // 128
    wstart = np.zeros(nchunks, np.int64)
    for g in range(nchunks):
        lo, hi = ts[g * 128], ts[g * 128 + 127]
        width = starts[hi + 1] - starts[lo]
        assert width <= W - 2, f"class window {width} too wide for chunk {g}"
        w0 = min(int(starts[lo]), N - W) & ~1
        wstart[g] = w0

    iotab = np.ascontiguousarray(
        np.broadcast_to(np.arange(C, dtype=np.float32), (128, C)))

    in_maps = []
    mrows = []
    for k in range(M):
        r0 = k * RPC
        g0 = r0 // 128
        mrow = np.ascontiguousarray(ms[r0:r0 + RPC].reshape(NCH, 128).T)
        trowf = np.ascontiguousarray(ts[r0:r0 + RPC].reshape(NCH, 128).T.astype(np.float32))
        xtw = np.concatenate(
            [xt_bf[:, wstart[g0 + ch]:wstart[g0 + ch] + W] for ch in range(NCH)], axis=1)
        eqm = np.concatenate(
            [(ts[r0 + ch * 128:r0 + (ch + 1) * 128, None]
              == ts[None, wstart[g0 + ch]:wstart[g0 + ch] + W]).astype(BF16)
             for ch in range(NCH)], axis=1)
        in_maps.append({
            "xt": xt_bf,
            "xtl": np.ascontiguousarray(xt_bf[:, r0:r0 + RPC]),
            "xtw": np.ascontiguousarray(xtw),
            "xsr": np.ascontiguousarray(x_bf[r0:r0 + RPC]),
            "mrow": mrow,
            "trow": trowf,
            "eqm": np.ascontiguousarray(eqm),
            "iotab": iotab,
        })
        mrows.append(mrow.astype(np.float64))
    return in_maps, mrows, cnt


def kernel(inputs, margin, targets):
    global _nc_cache, LAST_RESULTS
    in_maps, mrows, cnt = _prep(inputs, margin, targets)
    if _nc_cache is None:
        _nc_cache = _build_nc()
    res = run_bass_kernel_spmd(_nc_cache, in_maps, list(range(M)))
    LAST_RESULTS = res

    neg = 0.0
    V = np.zeros((C, 128), np.float64)
    for k in range(M):
        acc = res.results[k]["out_acc"].astype(np.float64)   # [128, 32]
        mr = mrows[k]                                        # [128, NCH]
        neg += acc[:, 0:8].sum()                             # sum relu(sim - m)
        neg += (mr * acc[:, 8:16]).sum()                     # m_i * cnt_i
        neg -= acc[:, 16:24].sum()                           # same-class relu part
        neg -= (mr * acc[:, 24:32]).sum()                    # m_i * same-class cnt
        V += res.results[k]["out_v"].astype(np.float64)

    pos = (cnt ** 2).sum() - (V ** 2).sum()
    loss = (pos + neg) / N
    return np.float32(loss)
